# revision 31
# baseline (speedup 1.0000x reference)
"""Trainium2 Bass kernel for nn_Network_67388036874689.

Data-parallel over batch: B=256 sharded as 32 samples on each of 8 cores;
all parameters replicated (host-precomposed).

Structure exploited (validated against the reference on host):
  - fog_of_war's greedy scan returns arange(B) -> the permutation is identity.
  - conv2d(3x3, pad=1) on [C, H, 1] spatial input only sees kernel column 1
    -> 1D conv over H with 3 taps.
  - Embedding (V=14) + pair-maxpool + conv compose into per-tap tables
    CW[kh] = pairmax_table @ conv_w[:, :, kh].T  (196 x 256), host-built.
    Device conv = one-hot(pair idx) matmuls against CW with +-1 shifts.
  - Conv bias folds into the following linear's bias (host).
  - The manipulator conv input is constant over h -> the 8192x256 manip
    linear collapses to 3 reduced 64x256 matrices (host-summed over h).

Precision: tables/linears in bf16 (host sim: 2/65536 token flips,
rel err ~1e-4); manipulator path f32/f32r; all psum accumulation f32.
"""

import numpy as np
import ml_dtypes
from contextlib import ExitStack

import concourse.bass as bass
import concourse.bacc as bacc
import concourse.mybir as mybir
import concourse.tile as tile
from concourse.masks import make_identity
from concourse.bass_utils import run_bass_kernel_spmd

F32 = mybir.dt.float32
F32R = mybir.dt.float32r
BF16 = mybir.dt.bfloat16
I32 = mybir.dt.int32
AF = mybir.ActivationFunctionType
ALU = mybir.AluOpType
AX = mybir.AxisListType

NCORES = 8
B = 256
BC = B // NCORES        # 32 samples per core
L = 256
V = 14
EMB = 512
H = L // 2              # 128 pooled positions
NPAIR = V * V           # 196
P0 = 112                # pair-table partition split: 112 + 84
P1 = NPAIR - P0
SW = H + 2              # 130: per-sample padded width in the one-hot tiles
OHW = BC * SW           # 4160
DEBUG_TAPS = False


def _dram_inputs(nc):
    t = {}

    def inp(name, shape, dt):
        t[name] = nc.dram_tensor(name, list(shape), dt, kind="ExternalInput").ap()

    inp("idxrowE", (1, BC * H), BF16)   # host: 14*x[:, 0::2] + x[:, 1::2], flat
    inp("cwE0", (P0, 768), BF16)     # enemy CW tables, col = kh*256 + o
    inp("cwE1", (P1, 768), BF16)
    inp("cwF0", (P0, 768), BF16)
    inp("cwF1", (P1, 768), BF16)
    inp("elw3", (256, 128 * 128), BF16)   # [o, (h, j)]
    inp("flw3", (256, 128 * 128), BF16)
    inp("mlwS", (64, 768), F32R)     # col = v*256 + j, v in (int, h0, hL)
    inp("wsumT", (128, 192), F32R)   # col = v*64 + o
    inp("mcb", (64,), F32)
    inp("elbe", (128,), F32)         # enemy lin bias + folded conv bias
    inp("flbe", (128,), F32)
    inp("mlb", (256,), F32)
    inp("f2w", (128, 14), F32)
    inp("f2b", (14,), F32)
    inp("foldT", (128, BC), F32R)   # foldT[32g+s, s] = 1: 4-group psum fold
    t["out"] = nc.dram_tensor("out", [BC, 14], F32, kind="ExternalOutput").ap()
    return t


def _tap(nc, io, name, ap):
    if not DEBUG_TAPS:
        return
    t = nc.dram_tensor("tap_" + name, list(ap.shape), ap.dtype,
                       kind="ExternalOutput").ap()
    io["tap_" + name] = t
    nc.gpsimd.dma_start(t, ap)


def build_kernel(nc, tc, ctx):
    io = _dram_inputs(nc)
    consts = ctx.enter_context(tc.tile_pool(name="consts", bufs=1))
    work = ctx.enter_context(tc.tile_pool(name="work", bufs=1))
    wpool = ctx.enter_context(tc.tile_pool(name="wstream", bufs=7))
    ohpool = ctx.enter_context(tc.tile_pool(name="ohpool", bufs=1))
    ppp = ctx.enter_context(tc.tile_pool(name="ppp", bufs=2, space="PSUM"))
    pconv = ctx.enter_context(tc.tile_pool(name="pconv", bufs=4, space="PSUM"))
    plin = ctx.enter_context(tc.tile_pool(name="plin", bufs=1, space="PSUM"))
    psm = ctx.enter_context(tc.tile_pool(name="psm", bufs=1, space="PSUM"))

    def ctile(shape, dt, tag):
        return consts.tile(shape, dt, tag=tag, name=tag)

    def wtile(shape, dt, tag):
        return work.tile(shape, dt, tag=tag, name=tag)

    # ---------------- constants & small weights ----------------
    identF = ctile([128, 128], F32, "identF")
    make_identity(nc, identF)
    iota_i = ctile([128, 1], I32, "iota_i")
    nc.gpsimd.iota(iota_i[:, :], pattern=[[0, 1]], base=0, channel_multiplier=1)
    iota_col = ctile([128, 1], F32, "iota_col")
    nc.vector.tensor_copy(iota_col[:, :], iota_i[:, :])
    ones_row = ctile([1, 128], BF16, "ones_row")
    nc.vector.memset(ones_row[:, :], 1.0)

    def bias_col(dram_vec, n, tag):
        col = ctile([n, 1], F32, tag)
        nc.gpsimd.dma_start(col[:, :], dram_vec)
        return col

    def bias_bcast(dram_vec, rows, width, tag):
        out = ctile([rows, width], F32, tag)
        nc.gpsimd.dma_start(out[:, :], dram_vec[None, :].partition_broadcast(rows))
        return out

    # enemy pair-index row first on the sync HWDGE ring (host-precomputed):
    # the whole front of the kernel needs it
    idxrowE = wtile([1, BC * H], BF16, "idxrowE")
    nc.sync.dma_start(idxrowE[:, :], io["idxrowE"])

    elbeB = bias_bcast(io["elbe"], BC, 128, "elbeB")
    flbeB = bias_bcast(io["flbe"], BC, 128, "flbeB")
    mlbB = bias_bcast(io["mlb"], BC, 256, "mlbB")
    f2bB = bias_bcast(io["f2b"], BC, 14, "f2bB")
    mcb_col = bias_col(io["mcb"], 64, "mcb")

    def load(name, shape, dt):
        t = ctile(shape, dt, name)
        nc.sync.dma_start(t[:, :], io[name])
        return t

    # All HBM loads go on the single sync HWDGE ring in exact consumption
    # order: ring FIFO means the small early loads fully drain before the
    # big weight streams start. (Splitting across rings lets the SDMA
    # engines' packet-granular round-robin starve the small-packet queue:
    # 1.5 KB vs 16 KB packets -> the conv tables took 13+ us to land.)
    # elw3/flw3: [o(256), (h,j)]; piece = [o-half(128), 64 h x 128 j] = 2 MB
    def stream_weights(dram):
        pieces = []
        for half in range(2):
            for hb in range(2):
                p = wpool.tile([128, 64 * 128], BF16, tag="wp", name="wp")
                nc.sync.dma_start(
                    p[:, :], dram[half * 128:(half + 1) * 128,
                                  hb * 8192:(hb + 1) * 8192])
                pieces.append(p)
        return pieces

    cwE0 = load("cwE0", [P0, 768], BF16)
    cwE1 = load("cwE1", [P1, 768], BF16)
    elwP = stream_weights(io["elw3"])
    wsumT = load("wsumT", [128, 192], F32R)
    mlwS = load("mlwS", [64, 768], F32R)
    cwF0 = load("cwF0", [P0, 768], BF16)
    cwF1 = load("cwF1", [P1, 768], BF16)
    flwP = stream_weights(io["flw3"])
    w2sb = load("f2w", [128, 14], F32)
    foldT = load("foldT", [128, BC], F32R)

    # dummy row for PE warm-up matmuls (HAM releases the clock gate only
    # after a sustained-busy window; the front of the kernel otherwise sits
    # idle waiting on DMAs and the whole enemy conv runs at 1.2 GHz)
    dumrow = ctile([1, 512], BF16, "dumrow")
    nc.vector.memset(dumrow[:, :], 0.5)

    def pe_warm(n):
        for _ in range(n):
            pp = ppp.tile([P0, 512], F32, tag="pp", name="warm")
            nc.tensor.matmul(pp[:, :], ones_row[:, 0:P0], dumrow[:, :],
                             start=True, stop=True)

    pe_warm(8)

    # ---------------- stage helpers ----------------
    def build_oh(idxrow, tag):
        """One-hot over the pair-idx row [1, 4096] (col s*128+h), padded
        layout: col s*130 + 1 + h holds [idx[s,h] == t]; cols s*130 and
        s*130+129 are zero (conv boundary)."""
        oh0 = ohpool.tile([P0, OHW], BF16, tag="oh0", name=f"oh0{tag}")
        oh1 = ohpool.tile([P1, OHW], BF16, tag="oh1", name=f"oh1{tag}")
        nc.vector.memset(oh0[:, 0:OHW:SW], 0.0)
        nc.vector.memset(oh0[:, SW - 1:OHW:SW], 0.0)
        nc.gpsimd.memset(oh1[:, 0:OHW:SW], 0.0)
        nc.gpsimd.memset(oh1[:, SW - 1:OHW:SW], 0.0)
        for blk in range(8):
            pp = ppp.tile([P0, 512], F32, tag="pp", name="pp")
            nc.tensor.matmul(pp[:, :], ones_row[:, 0:P0],
                             idxrow[:, blk * 512:(blk + 1) * 512],
                             start=True, stop=True)
            src = pp[:, :].rearrange("p (s w) -> p s w", w=128)
            dst0 = oh0[:, blk * 4 * SW:(blk + 1) * 4 * SW] \
                .rearrange("p (s w) -> p s w", w=SW)[:, :, 1:129]
            nc.vector.tensor_scalar(dst0, src, iota_col[0:P0, :], None,
                                    ALU.is_equal)
            dst1 = oh1[:, blk * 4 * SW:(blk + 1) * 4 * SW] \
                .rearrange("p (s w) -> p s w", w=SW)[:, :, 1:129]
            nc.vector.tensor_scalar(dst1, src[0:P1], float(P0),
                                    iota_col[0:P1, :], ALU.subtract,
                                    ALU.is_equal)
        return oh0, oh1

    def conv_apply(oh0, oh1, cw0, cw1, tag):
        """y[o, (s,h)] = sum_kh CW_kh[idx[h+kh-1], o]; acts as 2 halves
        [128 o', 32*128 (s,h)] bf16."""
        acts = [wtile([128, BC * H], BF16, f"acts{tag}{oc}") for oc in range(2)]
        for oc in range(2):
            for blk in range(8):
                cp = pconv.tile([128, 512], F32, tag="cp", name="cp")
                n = 0
                for cw, oh, npart in ((cw0, oh0, P0), (cw1, oh1, P1)):
                    for kh in range(3):
                        lhsT = cw[:, kh * 256 + oc * 128:
                                  kh * 256 + (oc + 1) * 128]
                        rhs = oh[:, blk * 4 * SW:(blk + 1) * 4 * SW] \
                            .rearrange("p (s w) -> p s w", w=SW)[:, :, kh:kh + 128]
                        nc.tensor.matmul(cp[:, :], lhsT, rhs,
                                         start=(n == 0), stop=(n == 5))
                        n += 1
                dst = acts[oc][:, blk * 512:(blk + 1) * 512]
                if blk % 2 == 0:
                    nc.scalar.activation(dst, cp[:, :], AF.Copy)
                else:
                    nc.vector.tensor_copy(dst, cp[:, :])
        return acts

    def big_linear(acts, pieces, tag):
        """lp[s, j] = sum_{o,h} acts[o][:, s*128+h] * W[(o,h), j].
        4x column-packed: 4 h-chunks run concurrently in disjoint 32-col
        PE array groups -> psum partition groups, folded by one matmul."""
        lpS = plin.tile([128, 128], F32, tag="lp", name=f"lpS{tag}")
        for c in range(256):
            half, h = divmod(c, 128)
            piece = pieces[half * 2 + h // 64]
            lhsT = acts[half][:, h:h + (BC - 1) * 128 + 1:128]
            rhs = piece[:, (h % 64) * 128:(h % 64 + 1) * 128]
            g = c % 4
            nc.tensor.matmul(lpS[32 * g:32 * (g + 1), :], lhsT, rhs,
                             start=(c < 4), stop=(c >= 252),
                             tile_position=(0, 32 * g))
        lpSs = wtile([128, 128], F32R, f"lpSs{tag}")
        nc.vector.tensor_copy(lpSs[:, :], lpS[:, :])
        lp2 = psm.tile([BC, 128], F32, tag="sm", name=f"lp2{tag}")
        nc.tensor.matmul(lp2[:, :], foldT[:, :], lpSs[:, :],
                         start=True, stop=True)
        return lp2

    # ---------------- enemy branch ----------------
    ohE0, ohE1 = build_oh(idxrowE, "E")
    actsE = conv_apply(ohE0, ohE1, cwE0, cwE1, "E")
    _tap(nc, io, "actsE0", actsE[0][:, :])
    lpE = big_linear(actsE, elwP, "E")

    logitsE = wtile([BC, 128], F32, "logitsE")
    nc.vector.tensor_tensor(logitsE[:, :], lpE[:, :], elbeB[:, :], ALU.add)
    _tap(nc, io, "logitsE", logitsE[:, :])
    nmxE = wtile([BC, 1], F32, "nmxE")
    nc.vector.reduce_max(nmxE[:, :], logitsE[:, :], AX.X, negate=True)
    ExE = wtile([BC, 128], F32, "ExE")
    nc.scalar.activation(ExE[:, :], logitsE[:, :], AF.Exp, bias=nmxE[:, :])
    smE = wtile([BC, 1], F32, "smE")
    nc.vector.reduce_sum(smE[:, :], ExE[:, :], AX.X)
    rsE = wtile([BC, 1], F32, "rsE")
    nc.vector.reciprocal(rsE[:, :], smE[:, :])
    eout = wtile([BC, 128], F32, "eout")
    nc.vector.tensor_scalar(eout[:, :], ExE[:, :], rsE[:, :], None, ALU.mult)

    tpv = psm.tile([128, BC], F32, tag="sm", name="tpv")
    nc.tensor.transpose(tpv[:, :], eout[:, :], identF[0:BC, 0:BC])
    vT = wtile([128, BC], F32R, "vT")
    nc.vector.tensor_copy(vT[:, :], tpv[:, :])
    _tap(nc, io, "vT", vT[:, :])

    # ---------------- manipulator ----------------
    cxs = {}
    for i, v in enumerate(("int", "h0", "hL")):
        cx = psm.tile([64, BC], F32, tag="sm", name=f"cx{v}")
        nc.tensor.matmul(cx[:, :], wsumT[:, i * 64:(i + 1) * 64], vT[:, :],
                         start=True, stop=True)
        cxs[v] = wtile([64, BC], F32R, f"cxs_{v}")
        nc.scalar.activation(cxs[v][:, :], cx[:, :], AF.Relu, bias=mcb_col[:, :])
    mp = psm.tile([BC, 256], F32, tag="sm", name="mp")
    for i, v in enumerate(("int", "h0", "hL")):
        nc.tensor.matmul(mp[:, :], cxs[v][:, :], mlwS[:, i * 256:(i + 1) * 256],
                         start=(i == 0), stop=(i == 2))
    # keep the PE busy through the (vector/scalar-bound) token stretch so the
    # HAM clock gate doesn't re-throttle before the friend conv
    pe_warm(14)
    m_sb = wtile([BC, 256], F32, "m_sb")
    nc.vector.tensor_tensor(m_sb[:, :], mp[:, :], mlbB[:, :], ALU.add)
    _tap(nc, io, "m", m_sb[:, :])

    # tokens = floor(|m|*100) mod 14; pair idx = 14*even + odd
    # floor via the 2^23 magic-number trick; mod 14 via 2 conditional subtracts
    tt = wtile([BC, 256], F32, "tt")
    nc.scalar.activation(tt[:, :], m_sb[:, :], AF.Abs, scale=100.0)
    fr = wtile([BC, 256], F32, "fr")
    nc.vector.tensor_scalar(fr[:, :], tt[:, :], 8388607.5, 8388608.0,
                            ALU.add, ALU.subtract)
    ti = wtile([BC, 256], F32, "ti")
    nc.vector.tensor_scalar(ti[:, :], fr[:, :], float(V), None, ALU.is_ge)
    t1 = wtile([BC, 256], F32, "t1")
    nc.vector.scalar_tensor_tensor(t1[:, :], ti[:, :], -float(V), fr[:, :],
                                   ALU.mult, ALU.add)
    t2 = wtile([BC, 256], F32, "t2")
    nc.vector.tensor_scalar(t2[:, :], t1[:, :], float(V), None, ALU.is_ge)
    tok = wtile([BC, 256], F32, "tok")
    nc.vector.scalar_tensor_tensor(tok[:, :], t2[:, :], -float(V), t1[:, :],
                                   ALU.mult, ALU.add)
    _tap(nc, io, "tok", tok[:, :])
    idxF = wtile([BC, H], BF16, "idxF")
    nc.vector.scalar_tensor_tensor(idxF[:, :], tok[:, 0:256:2], float(V),
                                   tok[:, 1:256:2], ALU.mult, ALU.add)
    idxrowF = wtile([1, BC * H], BF16, "idxrowF")
    nc.gpsimd.dma_start(idxrowF[:, :], idxF[:, :])

    # ---------------- friend branch ----------------
    ohF0, ohF1 = build_oh(idxrowF, "F")
    actsF = conv_apply(ohF0, ohF1, cwF0, cwF1, "F")
    lpF = big_linear(actsF, flwP, "F")
    fsb = wtile([BC, 128], F32, "fsb")
    nc.vector.tensor_tensor(fsb[:, :], lpF[:, :], flbeB[:, :], ALU.add)

    tpf = psm.tile([128, BC], F32, tag="sm", name="tpf")
    nc.tensor.transpose(tpf[:, :], fsb[:, :], identF[0:BC, 0:BC])
    fT = wtile([128, BC], F32, "fT")
    nc.vector.tensor_copy(fT[:, :], tpf[:, :])
    f2 = psm.tile([BC, 14], F32, tag="sm", name="f2")
    nc.tensor.matmul(f2[:, :], fT[:, :], w2sb[:, :], start=True, stop=True)
    logits = wtile([BC, 14], F32, "logits")
    nc.vector.tensor_tensor(logits[:, :], f2[:, :], f2bB[:, :], ALU.add)
    nmx = wtile([BC, 1], F32, "nmx")
    nc.vector.reduce_max(nmx[:, :], logits[:, :], AX.X, negate=True)
    ex = wtile([BC, 14], F32, "ex")
    nc.scalar.activation(ex[:, :], logits[:, :], AF.Exp, bias=nmx[:, :])
    sm = wtile([BC, 1], F32, "sm")
    nc.vector.reduce_sum(sm[:, :], ex[:, :], AX.X)
    rs = wtile([BC, 1], F32, "rs")
    nc.vector.reciprocal(rs[:, :], sm[:, :])
    outt = wtile([BC, 14], F32, "outt")
    nc.vector.tensor_scalar(outt[:, :], ex[:, :], rs[:, :], None, ALU.mult)
    nc.sync.dma_start(io["out"], outt[:, :])


_CACHE = {}


def _get_nc():
    if "nc" not in _CACHE:
        nc = bacc.Bacc("TRN2", target_bir_lowering=False, debug=False,
                       num_devices=NCORES)
        with tile.TileContext(nc) as tc:
            with ExitStack() as ctx:
                build_kernel(nc, tc, ctx)
        nc.compile()
        _CACHE["nc"] = nc
    return _CACHE["nc"]


def prep_inputs(inputs):
    """Host-side composition + shard. Returns list of 8 in_maps."""
    f32 = np.float32
    bf16 = ml_dtypes.bfloat16

    def cw_tables(emb, cw_full):
        emb = np.asarray(emb, f32)
        cw = np.ascontiguousarray(np.asarray(cw_full, f32)[:, :, :, 1])  # [O,I,3]
        t0, t1 = np.meshgrid(np.arange(V), np.arange(V), indexing="ij")
        table = np.maximum(emb[t0.ravel()], emb[t1.ravel()])            # [196,512]
        cwc = np.concatenate([table @ cw[:, :, kh].T for kh in range(3)],
                             axis=1).astype(bf16)                        # [196,768]
        return np.ascontiguousarray(cwc[:P0]), np.ascontiguousarray(cwc[P0:])

    cwE0, cwE1 = cw_tables(inputs["enemy_emb"], inputs["enemy_conv_w"])
    cwF0, cwF1 = cw_tables(inputs["friend_emb"], inputs["friend_conv_w"])

    elw = np.asarray(inputs["enemy_lin_w"], f32)
    flw = np.asarray(inputs["friend_lin1_w"], f32)
    elbe = (np.asarray(inputs["enemy_lin_b"], f32)
            + np.einsum("o,ohj->j", np.asarray(inputs["enemy_conv_b"], f32),
                        elw.reshape(256, 128, 128), optimize=True)).astype(f32)
    flbe = (np.asarray(inputs["friend_lin1_b"], f32)
            + np.einsum("o,ohj->j", np.asarray(inputs["friend_conv_b"], f32),
                        flw.reshape(256, 128, 128), optimize=True)).astype(f32)

    mcw = np.asarray(inputs["manip_conv_w"], f32)[:, :, :, 1]  # [64,128,3]
    s_int = mcw.sum(2)
    s12 = mcw[:, :, 1] + mcw[:, :, 2]
    s01 = mcw[:, :, 0] + mcw[:, :, 1]
    wsumT = np.concatenate([s_int.T, s12.T, s01.T], axis=1).astype(f32)  # [128,192]

    mlw3 = np.asarray(inputs["manip_lin_w"], f32).reshape(64, 128, 256)
    mlwS = np.concatenate([mlw3[:, 1:127].sum(1), mlw3[:, 0], mlw3[:, 127]],
                          axis=1).astype(f32)                            # [64,768]

    foldT = np.zeros((128, BC), f32)
    foldT[np.arange(128), np.arange(128) % BC] = 1.0

    common = {
        "cwE0": cwE0, "cwE1": cwE1, "cwF0": cwF0, "cwF1": cwF1,
        "elw3": np.ascontiguousarray(elw.reshape(256, 128 * 128)).astype(bf16),
        "flw3": np.ascontiguousarray(flw.reshape(256, 128 * 128)).astype(bf16),
        "mlwS": np.ascontiguousarray(mlwS),
        "wsumT": np.ascontiguousarray(wsumT),
        "mcb": np.ascontiguousarray(inputs["manip_conv_b"], f32),
        "elbe": elbe,
        "flbe": flbe,
        "mlb": np.ascontiguousarray(inputs["manip_lin_b"], f32),
        "f2w": np.ascontiguousarray(inputs["friend_lin2_w"], f32),
        "f2b": np.ascontiguousarray(inputs["friend_lin2_b"], f32),
        "foldT": foldT,
    }
    x = np.asarray(inputs["x"], np.int64)
    idxrow = (V * x[:, 0::2] + x[:, 1::2]).astype(bf16)   # [B, 128], ints < 196
    return [dict(common,
                 idxrowE=np.ascontiguousarray(
                     idxrow[c * BC:(c + 1) * BC].reshape(1, BC * H)))
            for c in range(NCORES)]


def kernel(**inputs):
    nc = _get_nc()
    in_maps = prep_inputs(inputs)
    res = run_bass_kernel_spmd(nc, in_maps, core_ids=list(range(NCORES)))
    return np.concatenate([r["out"] for r in res.results], axis=0)
